# revision 1
# baseline (speedup 1.0000x reference)
"""Trainium2 Bass kernel for nn_CrossAttention (B=4, C=256, H=W=48, heads=4).

Sharding: 8 cores = 4 batches x 2 query-row-halves. k/v replicated per
batch pair; attention queries split; no collectives. All heavy matmuls in
bf16 (fp32 PSUM accumulate); softmax exp on the scalar engine reading
PSUM scores transposed (keys on partitions) so the AV matmul contracts
keys directly and the softmax denominator rides along as a ones column.
"""

import numpy as np
import ml_dtypes

import concourse.bass as bass
import concourse.mybir as mybir
import concourse.tile as tile
from concourse import bacc
from concourse.bass_utils import run_bass_kernel_spmd

F32 = mybir.dt.float32
BF16 = mybir.dt.bfloat16

C = 256
H = W = 48
NK = H * W            # 2304 keys
KC = NK // 128        # 18 key chunks
HEADS = 4
HD = 64
ROWS_HALF = 24        # rows per core
NQ = ROWS_HALF * W    # 1152 query positions per core
QS = 384              # query slice (8 rows)
NQS = NQ // QS        # 3 slices
QROWS = ROWS_HALF + 2  # 26 rows incl halo
NQH = QROWS * W       # 1248
EPS = 1e-5

_CACHE = {}
import os as _os
_SKIP_NORM = bool(int(_os.environ.get("K_SKIP_NORM", "0")))
_SKIP_EXP = bool(int(_os.environ.get("K_SKIP_EXP", "0")))
_PHASE = int(_os.environ.get("K_PHASE", "3"))


def _build():
    nc = bacc.Bacc("TRN2", target_bir_lowering=False)
    dt = nc.dram_tensor
    qx = dt("qx", [C, NQH], BF16, kind="ExternalInput")
    kx = dt("kx", [C, NK], BF16, kind="ExternalInput")
    vx = dt("vx", [C, NK], BF16, kind="ExternalInput")
    wqT = dt("wqT", [C, C], BF16, kind="ExternalInput")
    wkT = dt("wkT", [C, C], BF16, kind="ExternalInput")
    wvT = dt("wvT", [C, C], BF16, kind="ExternalInput")
    wpT = dt("wpT", [C, C], BF16, kind="ExternalInput")
    pdg = dt("pdg", [18, 128, 128], BF16, kind="ExternalInput")
    tq1 = dt("tq1", [1, 2, 128], BF16, kind="ExternalInput")   # q bias (rank-1 lhsT)
    hq = dt("hq", [1, NQH], BF16, kind="ExternalInput")        # halo row mask
    tkv = dt("tkv", [2, 128, 1], F32, kind="ExternalInput")    # k bias per-partition
    tvb = dt("tvb", [128, 264], F32, kind="ExternalInput")     # v bias in vf layout
    tpv = dt("tpv", [2, 128, 1], F32, kind="ExternalInput")    # pe bias
    tjv = dt("tjv", [2, 128, 1], F32, kind="ExternalInput")    # proj bias
    o = dt("o", [C, NQ], F32, kind="ExternalOutput")

    with tile.TileContext(nc) as tc:
        with (
            tc.tile_pool(name="wp", bufs=1) as wp,
            tc.tile_pool(name="inp", bufs=1) as inp,
            tc.tile_pool(name="feat", bufs=1) as feat,
            tc.tile_pool(name="vfp", bufs=18) as vfp,
            tc.tile_pool(name="et", bufs=40) as etp,
            tc.tile_pool(name="small", bufs=3) as smp,
            tc.tile_pool(name="ps_s", bufs=2, space="PSUM") as ps_s,
            tc.tile_pool(name="ps_w", bufs=2, space="PSUM") as ps_w,
        ):
            # ---- constants / weights to SBUF ----
            w_q = wp.tile([128, 2, C], BF16, tag="wq")
            w_k = wp.tile([128, 2, C], BF16, tag="wk")
            w_v = wp.tile([128, 2, C], BF16, tag="wv")
            w_p = wp.tile([128, 2, C], BF16, tag="wpj")
            for t, src in ((w_q, wqT), (w_k, wkT), (w_v, wvT), (w_p, wpT)):
                nc.sync.dma_start(out=t[:], in_=src[:].rearrange("(a p) n -> p a n", p=128))
            w_d = wp.tile([128, 18, 128], BF16, tag="wd")
            nc.sync.dma_start(out=w_d[:], in_=pdg[:].rearrange("t p n -> p t n"))
            tq_sb = wp.tile([1, 2, 128], BF16, tag="tq")
            nc.sync.dma_start(out=tq_sb[:], in_=tq1[:])
            hq_sb = wp.tile([1, NQH], BF16, tag="hq")
            nc.sync.dma_start(out=hq_sb[:], in_=hq[:])
            tk_sb = wp.tile([128, 2, 1], F32, tag="tk")
            nc.sync.dma_start(out=tk_sb[:], in_=tkv[:].rearrange("a p x -> p a x"))
            tv_sb = wp.tile([128, 264], F32, tag="tv")
            nc.sync.dma_start(out=tv_sb[:], in_=tvb[:])
            tp_sb = wp.tile([128, 2, 1], F32, tag="tp")
            nc.sync.dma_start(out=tp_sb[:], in_=tpv[:].rearrange("a p x -> p a x"))
            tj_sb = wp.tile([128, 2, 1], F32, tag="tj")
            nc.sync.dma_start(out=tj_sb[:], in_=tjv[:].rearrange("a p x -> p a x"))

            # ---- inputs to SBUF ----
            k_sb = inp.tile([128, 2, NK], BF16, tag="k")
            v_sb = inp.tile([128, 2, NK], BF16, tag="v")
            q_sb = inp.tile([128, 2, NQH], BF16, tag="q")
            nc.sync.dma_start(out=k_sb[:], in_=kx[:].rearrange("(a p) n -> p a n", p=128))
            nc.sync.dma_start(out=v_sb[:], in_=vx[:].rearrange("(a p) n -> p a n", p=128))
            nc.sync.dma_start(out=q_sb[:], in_=qx[:].rearrange("(a p) n -> p a n", p=128))

            # ---- vf: position-major value features, 18 tiles [128, 4, 66] ----
            # per head h: cols [v(64) | 1 | pad]
            vf = []
            for pc in range(KC):
                vt = vfp.tile([128, 4, 66], BF16, tag="vf")
                nc.vector.memset(vt[:], 1.0)
                ps = ps_w.tile([128, 512], F32, tag="w")
                for ci in range(2):
                    nc.tensor.matmul(
                        ps[:, 0:C],
                        v_sb[:, ci, pc * 128:(pc + 1) * 128],
                        w_v[:, ci, :],
                        start=(ci == 0), stop=(ci == 1),
                    )
                psv = ps[:, 0:C].rearrange("p (h d) -> p h d", h=4)
                tvv = tv_sb[:].rearrange("p (h f) -> p h f", h=4)
                nc.vector.tensor_add(vt[:, :, 0:64], psv[:], tvv[:, :, 0:64])
                vf.append(vt)

            # ---- kf: channel-major key features [128, 2, NK] bf16 ----
            kf = feat.tile([128, 2, NK], BF16, tag="kf")
            for co in range(2):
                for n0 in range(0, NK, 512):
                    nn = min(512, NK - n0)
                    ps = ps_w.tile([128, 512], F32, tag="w")
                    for ci in range(2):
                        nc.tensor.matmul(
                            ps[:, 0:nn],
                            w_k[:, ci, co * 128:(co + 1) * 128],
                            k_sb[:, ci, n0:n0 + nn],
                            start=(ci == 0), stop=(ci == 1),
                        )
                    nc.vector.tensor_scalar(
                        kf[:, co, n0:n0 + nn], ps[:, 0:nn],
                        tk_sb[:, co, :], None, mybir.AluOpType.add,
                    )

            # ---- qf: channel-major query features (scaled), with halo rows ----
            qf = feat.tile([128, 2, NQH], BF16, tag="qf")
            for co in range(2):
                for n0 in range(0, NQH, 512):
                    nn = min(512, NQH - n0)
                    ps = ps_w.tile([128, 512], F32, tag="w")
                    for ci in range(2):
                        nc.tensor.matmul(
                            ps[:, 0:nn],
                            w_q[:, ci, co * 128:(co + 1) * 128],
                            q_sb[:, ci, n0:n0 + nn],
                            start=(ci == 0), stop=False,
                        )
                    # masked bias: qf += tq[c] * hmask[n]  (rank-1)
                    nc.tensor.matmul(
                        ps[:, 0:nn],
                        tq_sb[:, co, :],
                        hq_sb[:, n0:n0 + nn],
                        start=False, stop=True,
                    )
                    nc.vector.tensor_copy(qf[:, co, n0:n0 + nn], ps[:, 0:nn])

            qfr = qf[:].rearrange("p a (r w) -> p a r w", w=W)

            # ---- attention + pe + proj, software-pipelined across q slices:
            # while ACT runs exp for slice si, PE runs AV/pe/proj of si-1.
            def emit_s_group(st, t, h):
                hp, par = h // 2, h % 2
                rs = slice(par * 64, par * 64 + 64)
                s = ps_s.tile([128, 3, 512], F32, tag="s")
                for i in range(3):
                    kc = t * 3 + i
                    nc.tensor.matmul(
                        s[:, i, 0:QS],
                        kf[rs, hp, kc * 128:(kc + 1) * 128],
                        qf[rs, hp, st["q0"]:st["q0"] + QS],
                        start=True, stop=True,
                    )
                et = etp.tile([128, 3, QS], BF16, tag="et")
                if _SKIP_EXP:
                    nc.vector.tensor_copy(et[:], s[:, :, 0:QS])
                else:
                    nc.scalar.activation(et[:], s[:, :, 0:QS],
                                         mybir.ActivationFunctionType.Exp)
                st["ets"][t][h] = et

            def emit_av_head(st, h):
                y = ps_w.tile([128, 512], F32, tag="w")
                for t in range(6):
                    for i in range(3):
                        kc = t * 3 + i
                        nc.tensor.matmul(
                            y[0:65, 0:QS], vf[kc][:, h, 0:65],
                            st["ets"][t][h][:, i, :],
                            start=(kc == 0), stop=(kc == KC - 1),
                        )
                st["ys"][h] = y

            def emit_norm(st, pair):
                ys = [st["ys"][pair * 2], st["ys"][pair * 2 + 1]]
                ynt = smp.tile([128, QS], BF16, tag="yn")
                if _SKIP_NORM:
                    nc.vector.tensor_copy(ynt[0:64, :], ys[0][0:64, 0:QS])
                    nc.vector.tensor_copy(ynt[64:128, :], ys[1][0:64, 0:QS])
                else:
                    rr = smp.tile([1, 2, QS], F32, tag="rr")
                    rq = smp.tile([128, 2, QS], F32, tag="rq")
                    for par in range(2):
                        nc.vector.reciprocal(rr[:, par, :], ys[par][64:65, 0:QS])
                    nc.gpsimd.partition_broadcast(rq[:], rr[:])
                    nc.vector.tensor_mul(ynt[0:64, :], ys[0][0:64, 0:QS], rq[0:64, 0, :])
                    nc.vector.tensor_mul(ynt[64:128, :], ys[1][0:64, 0:QS], rq[64:128, 1, :])
                st["yn"][pair] = ynt

            def emit_tail(st):
                r0, si = st["r0"], st["si"]
                yt = [None, None]
                for ch in range(2):
                    pe = ps_w.tile([128, 512], F32, tag="w")
                    pev = pe[:, 0:QS].rearrange("p (r w) -> p r w", w=W)
                    first = True
                    for ti, (di, dj) in enumerate(
                        (di, dj) for di in (-1, 0, 1) for dj in (-1, 0, 1)
                    ):
                        j0o, j0i = max(0, -dj), max(0, dj)
                        ncol = W - abs(dj)
                        nc.tensor.matmul(
                            pev[:, :, j0o:j0o + ncol],
                            w_d[:, ti * 2 + ch, :],
                            qfr[:, ch, r0 + 1 + di:r0 + 9 + di, j0i:j0i + ncol],
                            start=first, stop=(ti == 8),
                        )
                        first = False
                    ytt = smp.tile([128, QS], BF16, tag="yt")
                    nc.vector.scalar_tensor_tensor(
                        out=ytt[:], in0=pe[:, 0:QS], scalar=tp_sb[:, ch, :],
                        in1=st["yn"][ch][:], op0=mybir.AluOpType.add,
                        op1=mybir.AluOpType.add,
                    )
                    yt[ch] = ytt
                ob = smp.tile([128, 2, QS], F32, tag="ob")
                for co in range(2):
                    pj = ps_w.tile([128, 512], F32, tag="w")
                    for ci in range(2):
                        nc.tensor.matmul(
                            pj[:, 0:QS],
                            w_p[:, ci, co * 128:(co + 1) * 128],
                            yt[ci][:],
                            start=(ci == 0), stop=(ci == 1),
                        )
                    nc.vector.tensor_scalar(
                        ob[:, co, :], pj[:, 0:QS], tj_sb[:, co, :], None,
                        mybir.AluOpType.add,
                    )
                nc.sync.dma_start(
                    out=o[:].rearrange("(a p) n -> p a n", p=128)[:, :, si * QS:(si + 1) * QS],
                    in_=ob[:],
                )

            FIRE = {4: lambda st: emit_av_head(st, 0),
                    8: lambda st: emit_av_head(st, 1),
                    12: lambda st: emit_norm(st, 0),
                    16: lambda st: emit_av_head(st, 2),
                    20: lambda st: emit_av_head(st, 3),
                    24: lambda st: emit_norm(st, 1)}

            prev = None
            for si in range(NQS + 1):
                cur = None
                if si < NQS and _PHASE >= 2:
                    cur = {"si": si, "q0": 48 + si * QS, "r0": si * (QS // W),
                           "ets": [[None] * HEADS for _ in range(6)],
                           "ys": [None] * 4, "yn": [None, None]}
                    g = 0
                    for t in range(6):
                        for h in range(HEADS):
                            emit_s_group(cur, t, h)
                            g += 1
                            if prev is not None and g in FIRE and _PHASE >= 3:
                                FIRE[g](prev)
                    if prev is not None and _PHASE >= 3:
                        emit_tail(prev)
                elif si == NQS and _PHASE >= 3:
                    for g in (4, 8, 12, 16, 20, 24):
                        FIRE[g](prev)
                    emit_tail(prev)
                prev = cur
    nc.compile()
    return nc


def _prep(inputs):
    """Host-side: fold BN into weights, build per-core input maps."""
    f64 = np.float64
    def fold(w, g, b, m, v):
        s = g.astype(f64) / np.sqrt(v.astype(f64) + EPS)
        return w.astype(f64) * s[:, None], b.astype(f64) - m.astype(f64) * s

    wq, tq = fold(inputs["wq_w"], inputs["wq_g"], inputs["wq_b"], inputs["wq_m"], inputs["wq_v"])
    wk, tk = fold(inputs["wk_w"], inputs["wk_g"], inputs["wk_b"], inputs["wk_m"], inputs["wk_v"])
    wv, tv = fold(inputs["wv_w"], inputs["wv_g"], inputs["wv_b"], inputs["wv_m"], inputs["wv_v"])
    wp, tj = fold(inputs["proj_w"], inputs["proj_g"], inputs["proj_b"], inputs["proj_m"], inputs["proj_v"])
    scale = 1.0 / np.sqrt(HD)
    wq, tq = wq * scale, tq * scale
    s_pe = inputs["pe_g"].astype(f64) / np.sqrt(inputs["pe_v"].astype(f64) + EPS)
    tp = inputs["pe_b"].astype(f64) - inputs["pe_m"].astype(f64) * s_pe
    w9 = inputs["pe_w"].astype(f64).reshape(C, 9) * s_pe[:, None] / scale  # pe sees unscaled qf

    bf = ml_dtypes.bfloat16
    pdg = np.zeros((18, 128, 128), dtype=bf)
    for tap in range(9):
        for ch in range(2):
            np.fill_diagonal(pdg[tap * 2 + ch], w9[ch * 128:(ch + 1) * 128, tap].astype(bf))

    tvb = np.zeros((128, 264), dtype=np.float32)
    tvv = tv.astype(np.float32).reshape(4, 64)
    for h in range(4):
        tvb[:, h * 66: h * 66 + 64] = tvv[h][None, :]

    common = {
        "wqT": wq.T.astype(bf), "wkT": wk.T.astype(bf),
        "wvT": wv.T.astype(bf), "wpT": wp.T.astype(bf),
        "pdg": pdg,
        "tq1": tq.astype(bf).reshape(1, 2, 128),
        "tkv": tk.astype(np.float32).reshape(2, 128, 1),
        "tvb": tvb,
        "tpv": tp.astype(np.float32).reshape(2, 128, 1),
        "tjv": tj.astype(np.float32).reshape(2, 128, 1),
    }

    q = inputs["q"].astype(np.float32).reshape(4, C, H, W)
    k = inputs["k"].astype(np.float32).reshape(4, C, NK)
    v = inputs["v"].astype(np.float32).reshape(4, C, NK)

    in_maps = []
    for c in range(8):
        b, half = c // 2, c % 2
        r0 = half * ROWS_HALF
        qh = np.zeros((C, QROWS, W), dtype=np.float32)
        hm = np.zeros((QROWS,), dtype=np.float32)
        lo, hi = max(0, r0 - 1), min(H, r0 + ROWS_HALF + 1)
        qh[:, lo - (r0 - 1):lo - (r0 - 1) + (hi - lo)] = q[b, :, lo:hi]
        hm[lo - (r0 - 1):lo - (r0 - 1) + (hi - lo)] = 1.0
        m = dict(common)
        m["qx"] = qh.reshape(C, NQH).astype(bf)
        m["hq"] = np.repeat(hm, W).reshape(1, NQH).astype(bf)
        m["kx"] = k[b].astype(bf)
        m["vx"] = v[b].astype(bf)
        in_maps.append(m)
    return in_maps


def _get_nc():
    if "nc" not in _CACHE:
        _CACHE["nc"] = _build()
    return _CACHE["nc"]


def run_cores(in_maps, trace=False):
    return run_bass_kernel_spmd(_get_nc(), in_maps, core_ids=list(range(8)), trace=trace)


def assemble(results):
    out = np.empty((4, C, H, W), dtype=np.float32)
    for c in range(8):
        b, half = c // 2, c % 2
        out[b, :, half * ROWS_HALF:(half + 1) * ROWS_HALF, :] = (
            results[c]["o"].reshape(C, ROWS_HALF, W)
        )
    return out


def kernel(**inputs):
    in_maps = _prep(inputs)
    res = run_cores(in_maps)
    return assemble(res.results)

